# revision 2
# baseline (speedup 1.0000x reference)
"""GQA attention (RoPE + ALiBi + causal) Bass kernel for Trainium2, 8 NeuronCores.

Sharding: core (b, g) = batch b in {0,1} x kv-group g in {0..3}; each core computes
its 4 query heads' attention for its batch and a partial output projection
(row-parallel wo); host sums the 4 group partials per batch.

Device dataflow (all matmuls in float32r = TF32-rate, fp32 accumulate):
  qT[hd,s] = wqT.T @ xT       (projection; weights host-pretransposed, rope-deinterleaved)
  RoPE on qT/kT via cos/sin tables (DVE), half-swapped layout
  per (head h, 512-q-window w, 128-kv-tile u):
    scoresT[kv,q] = kT_u.T-contract qT  (PSUM)
    P' = exp(scale*scores + bias_p) in 128-q chunks (ACT, per-partition bias)
      bias_p = slope*(k0 + p - (q0s + 96)) -- the -slope*q part of ALiBi cancels in
      softmax, so only a per-kv-position bias remains; per-chunk recentering keeps
      exp in fp32 range. Diagonal chunks get a 0/1 causal mask multiply.
    outT[hd,q] += V_u.T @ P'  ;  den += ones.T @ P'   (PSUM accumulate)
  attn_outT = outT * (1/den)  (reciprocal_approx_fast)
  partial[q,d] = sum_h attn_outT_h.T @ woT_h          (PSUM accumulate over heads)
"""
import math
from contextlib import ExitStack

import numpy as np

import concourse.bass as bass
import concourse.bacc as bacc
import concourse.tile as tile
from concourse import mybir
from concourse.bass_utils import run_bass_kernel_spmd

F32 = mybir.dt.float32
F32R = mybir.dt.float32r

B, S, D = 2, 2048, 2048
H, KV, HD, REP = 16, 4, 128, 4
NH = 4                     # heads per core
NW = S // 512              # q-windows
ND = D // 128              # d_in tiles
NU = S // 128              # kv tiles
SCALE = 1.0 / math.sqrt(HD)


def build():
    nc = bacc.Bacc(None)
    xT_d = nc.dram_tensor("xT", [D, S], F32R, kind="ExternalInput")
    wq_d = nc.dram_tensor("wqT", [D, NH * HD], F32R, kind="ExternalInput")
    wk_d = nc.dram_tensor("wkT", [D, HD], F32R, kind="ExternalInput")
    wv_d = nc.dram_tensor("wvT", [D, HD], F32R, kind="ExternalInput")
    wo_d = nc.dram_tensor("woT", [NH * HD, D], F32R, kind="ExternalInput")
    cosF_d = nc.dram_tensor("cosF", [128, S], F32R, kind="ExternalInput")
    sinF_d = nc.dram_tensor("sinF", [128, S], F32R, kind="ExternalInput")
    biasb_d = nc.dram_tensor("biasb", [128, NH * 16], F32, kind="ExternalInput")
    cmask_d = nc.dram_tensor("cmask", [128, 128], F32R, kind="ExternalInput")
    ident_d = nc.dram_tensor("ident", [128, 128], F32, kind="ExternalInput")
    ones_d = nc.dram_tensor("ones", [128, 128], F32R, kind="ExternalInput")
    part_d = nc.dram_tensor("part", [S, D], F32, kind="ExternalOutput")

    PSUM = bass.MemorySpace.PSUM

    with tile.TileContext(nc) as tc:
        with ExitStack() as ctx:
            consts = ctx.enter_context(tc.tile_pool(name="consts", bufs=1))
            persist = ctx.enter_context(tc.tile_pool(name="persist", bufs=1))

            cosF = consts.tile([128, S], F32R, tag="cosF")
            sinF = consts.tile([128, S], F32R, tag="sinF")
            biasb = consts.tile([128, NH * 16], F32, tag="biasb")
            cmask = consts.tile([128, 128], F32R, tag="cmask")
            ident = consts.tile([128, 128], F32, tag="ident")
            ones = consts.tile([128, 128], F32R, tag="ones")
            nc.sync.dma_start(cosF[:], cosF_d[:])
            nc.sync.dma_start(sinF[:], sinF_d[:])
            nc.sync.dma_start(biasb[:], biasb_d[:])
            nc.sync.dma_start(cmask[:], cmask_d[:])
            nc.sync.dma_start(ident[:], ident_d[:])
            nc.sync.dma_start(ones[:], ones_d[:])

            qT = [persist.tile([128, S], F32R, tag=f"qT{h}", name=f"qT{h}") for h in range(NH)]
            kT = persist.tile([128, S], F32R, tag="kT")
            vnat = persist.tile([128, S], F32R, tag="vnat")
            attn = [persist.tile([128, S], F32R, tag=f"attn{h}", name=f"attn{h}") for h in range(NH)]

            # ---------------- phase 1: Q/K/V projections (+ per-window RoPE) ---------
            with tc.tile_pool(name="wqkv", bufs=1) as wpool, \
                 tc.tile_pool(name="xsl", bufs=6) as xpool, \
                 tc.tile_pool(name="vtmp", bufs=1) as vpool, \
                 tc.tile_pool(name="rope", bufs=2) as rp, \
                 tc.tile_pool(name="pps", bufs=1, space=PSUM) as pps:
                wq_sb = wpool.tile([128, ND, NH * HD], F32R, tag="wq")
                wk_sb = wpool.tile([128, ND, HD], F32R, tag="wk")
                wv_sb = wpool.tile([128, ND, HD], F32R, tag="wv")
                nc.sync.dma_start(wq_sb[:], wq_d.rearrange("(t p) o -> p t o", p=128))
                nc.sync.dma_start(wk_sb[:], wk_d.rearrange("(t p) o -> p t o", p=128))
                nc.sync.dma_start(wv_sb[:], wv_d.rearrange("(t p) o -> p t o", p=128))
                vT = vpool.tile([128, S], F32, tag="vT")

                for w in range(NW):
                    sl = slice(w * 512, (w + 1) * 512)
                    pq = [pps.tile([128, 512], F32, tag=f"pq{h}", name=f"pq{h}") for h in range(NH)]
                    pk = pps.tile([128, 512], F32, tag="pk")
                    pv = pps.tile([128, 512], F32, tag="pv")
                    for d in range(ND):
                        xs = xpool.tile([128, 512], F32R, tag="x")
                        nc.sync.dma_start(xs[:], xT_d[d * 128:(d + 1) * 128, sl])
                        st, sp = (d == 0), (d == ND - 1)
                        for h in range(NH):
                            nc.tensor.matmul(pq[h][:], wq_sb[:, d, h * 128:(h + 1) * 128],
                                             xs[:], start=st, stop=sp)
                        nc.tensor.matmul(pk[:], wk_sb[:, d, :], xs[:], start=st, stop=sp)
                        nc.tensor.matmul(pv[:], wv_sb[:, d, :], xs[:], start=st, stop=sp)
                    for h in range(NH):
                        nc.scalar.copy(qT[h][:, sl], pq[h][:])
                    nc.scalar.copy(kT[:, sl], pk[:])
                    nc.vector.tensor_copy(vT[:, sl], pv[:])

                    # RoPE on this window's q/k slices: out = cosF*z + sinF*swap(z)
                    for tgt in [kT] + qT:
                        qb = rp.tile([128, 512], F32R, tag="qb")
                        nc.sync.dma_start(qb[0:64, :], tgt[64:128, sl])
                        nc.sync.dma_start(qb[64:128, :], tgt[0:64, sl])
                        t1 = rp.tile([128, 512], F32R, tag="t1")
                        nc.vector.tensor_mul(t1[:], tgt[:, sl], cosF[:, sl])
                        nc.vector.tensor_mul(qb[:], qb[:], sinF[:, sl])
                        nc.vector.tensor_add(tgt[:, sl], t1[:], qb[:])

                # V transpose: vT [hd, s] -> vnat [s(part), hd]
                for u in range(NU):
                    tp = pps.tile([128, 128], F32, tag=f"tp{u % 2}")
                    nc.tensor.transpose(tp[:], vT[:, u * 128:(u + 1) * 128], ident[:])
                    nc.scalar.copy(vnat[:, u * 128:(u + 1) * 128], tp[:])

            # ---------------- phase 2: attention + output projection ----------------
            with tc.tile_pool(name="sp", bufs=3, space=PSUM) as sp, \
                 tc.tile_pool(name="dp", bufs=1, space=PSUM) as dp, \
                 tc.tile_pool(name="op", bufs=2, space=PSUM) as op, \
                 tc.tile_pool(name="ojp", bufs=2, space=PSUM) as ojp, \
                 tc.tile_pool(name="Pp", bufs=5) as Pp, \
                 tc.tile_pool(name="ep", bufs=3) as ep, \
                 tc.tile_pool(name="wop", bufs=1) as wop, \
                 tc.tile_pool(name="ostg", bufs=4) as ostg:
                wo_sb = wop.tile([128, NH, D], F32R, tag="wo")
                nc.sync.dma_start(wo_sb[:], wo_d.rearrange("(h p) o -> p h o", p=128))

                for w in range(NW):
                    qsl = slice(w * 512, (w + 1) * 512)
                    U = 4 * (w + 1)
                    for h in range(NH):
                        o_ps = op.tile([128, 512], F32, tag="o")
                        d_ps = dp.tile([128, 512], F32, tag="den")
                        pend = None  # (Pt, n0, first?) pipelined by one kv-tile
                        for u in range(U):
                            i0 = max(0, u - 4 * w)
                            n0 = 128 * i0
                            s_ps = sp.tile([128, 512], F32, tag="s")
                            nc.tensor.matmul(
                                s_ps[:, n0:512],
                                kT[:, u * 128:(u + 1) * 128],
                                qT[h][:, w * 512 + n0:(w + 1) * 512],
                                start=True, stop=True)
                            Pt = Pp.tile([128, 512], F32R, tag="P")
                            for i in range(i0, 4):
                                t = 4 * w + i - u
                                csl = slice(i * 128, (i + 1) * 128)
                                nc.scalar.activation(
                                    Pt[:, csl], s_ps[:, csl],
                                    mybir.ActivationFunctionType.Exp,
                                    bias=biasb[:, h * 16 + t:h * 16 + t + 1],
                                    scale=SCALE)
                                if t == 0:
                                    nc.vector.tensor_mul(Pt[:, csl], Pt[:, csl], cmask[:])
                            if pend is not None:
                                pPt, pn0, pu = pend
                                nc.tensor.matmul(o_ps[:, pn0:512],
                                                 vnat[:, pu * 128:(pu + 1) * 128],
                                                 pPt[:, pn0:512],
                                                 start=(pu == 0), stop=False)
                                nc.tensor.matmul(d_ps[:, pn0:512], ones[:],
                                                 pPt[:, pn0:512],
                                                 start=(pu == 0), stop=False)
                            pend = (Pt, n0, u)
                        pPt, pn0, pu = pend
                        nc.tensor.matmul(o_ps[:, pn0:512],
                                         vnat[:, pu * 128:(pu + 1) * 128],
                                         pPt[:, pn0:512],
                                         start=(pu == 0), stop=True)
                        nc.tensor.matmul(d_ps[:, pn0:512], ones[:],
                                         pPt[:, pn0:512],
                                         start=(pu == 0), stop=True)
                        rec = ep.tile([128, 512], F32, tag="rec")
                        nc.vector.reciprocal_approx_fast(rec[:], d_ps[:])
                        nc.vector.tensor_mul(attn[h][:, qsl], o_ps[:], rec[:])

                    # output projection for this window's 4 q-tiles
                    for mq in range(4):
                        m = 4 * w + mq
                        for dg in range(2):
                            po = [ojp.tile([128, 512], F32, tag="oj", name=f"po{_k}") for _k in range(2)]
                            for h in range(NH):
                                for k2 in range(2):
                                    dwin = dg * 2 + k2
                                    nc.tensor.matmul(
                                        po[k2][:],
                                        attn[h][:, m * 128:(m + 1) * 128],
                                        wo_sb[:, h, dwin * 512:(dwin + 1) * 512],
                                        start=(h == 0), stop=(h == NH - 1))
                            for k2 in range(2):
                                dwin = dg * 2 + k2
                                so = ostg.tile([128, 512], F32, tag="so")
                                nc.vector.tensor_copy(so[:], po[k2][:])
                                nc.sync.dma_start(
                                    part_d[m * 128:(m + 1) * 128,
                                           dwin * 512:(dwin + 1) * 512], so[:])
    nc.finalize()
    return nc


_NC_CACHE = {}


def _get_nc():
    if "nc" not in _NC_CACHE:
        _NC_CACHE["nc"] = build()
    return _NC_CACHE["nc"]


def _host_prep(x, alibi_bias, wq, wk, wv, wo):
    """Build per-core input maps (shard + transpose + rope tables + bias tables)."""
    x = np.asarray(x, np.float32)
    alibi_bias = np.asarray(alibi_bias, np.float32)
    wq = np.asarray(wq, np.float32)
    wk = np.asarray(wk, np.float32)
    wv = np.asarray(wv, np.float32)
    wo = np.asarray(wo, np.float32)

    slopes = alibi_bias[0, :, 0, 1].copy()        # [H]; alibi[0,h,0,1] = slope_h

    inv_freq = 1.0 / (10000.0 ** (np.arange(0, HD, 2, dtype=np.float32) / HD))
    t = np.arange(S, dtype=np.float32)
    freqs = np.outer(t, inv_freq)                 # [S, 64]
    cos = np.cos(freqs).astype(np.float32).T      # [64, S]
    sin = np.sin(freqs).astype(np.float32).T
    cosF = np.ascontiguousarray(np.concatenate([cos, cos], 0))     # [128, S]
    sinF = np.ascontiguousarray(np.concatenate([-sin, sin], 0))

    perm = np.concatenate([np.arange(0, HD, 2), np.arange(1, HD, 2)])
    p_ar = np.arange(128, dtype=np.float32)
    cmask = (p_ar[:, None] <= p_ar[None, :]).astype(np.float32)
    ident = np.eye(128, dtype=np.float32)
    ones = np.ones((128, 128), np.float32)

    xTs = [np.ascontiguousarray(x[b].T) for b in range(B)]
    in_maps = []
    for core in range(8):
        b, g = divmod(core, KV)
        wq_g = wq[4 * g * HD:(4 * g + 4) * HD].reshape(NH, HD, D)[:, perm, :]
        wqT = np.ascontiguousarray(wq_g.reshape(NH * HD, D).T)
        wkT = np.ascontiguousarray(wk[g * HD:(g + 1) * HD][perm].T)
        wvT = np.ascontiguousarray(wv[g * HD:(g + 1) * HD].T)
        woT = np.ascontiguousarray(wo[:, 4 * g * HD:(4 * g + 4) * HD].T)
        biasb = np.zeros((128, NH * 16), np.float32)
        for h in range(NH):
            sl = slopes[4 * g + h]
            for tt in range(16):
                biasb[:, h * 16 + tt] = sl * (p_ar - 96.0 - 128.0 * tt)
        in_maps.append({
            "xT": xTs[b], "wqT": wqT, "wkT": wkT, "wvT": wvT, "woT": woT,
            "cosF": cosF, "sinF": sinF, "biasb": biasb, "cmask": cmask,
            "ident": ident, "ones": ones,
        })
    return in_maps


def kernel(x, mask, alibi_bias, wq, wk, wv, wo, _trace=False, _trace_kwargs=None):
    nc = _get_nc()
    in_maps = _host_prep(x, alibi_bias, wq, wk, wv, wo)
    res = run_bass_kernel_spmd(nc, in_maps, list(range(8)), trace=_trace,
                               **(_trace_kwargs or {}))
    parts = [res.results[c]["part"] for c in range(8)]
    out = np.stack([
        parts[0] + parts[1] + parts[2] + parts[3],
        parts[4] + parts[5] + parts[6] + parts[7],
    ]).astype(np.float32)
    if _trace:
        return out, res
    return out
